# revision 27
# baseline (speedup 1.0000x reference)
"""Trainium2 Bass kernel for nn_NearestEmbedding (vq_codebook).

reference:
  xn  = BatchNorm1d(x)   (training mode, biased batch stats)
  out = weight[argmin_k ||xn - weight_k||^2]

Strategy (8 NeuronCores, data-parallel over N):
  - each core takes an x shard [2048, 256]; weight/gamma/beta replicated
  - BN batch stats via on-device AllReduce of per-core [sum, sumsq]
  - argmin via maximization of v = 2*xn.w_k - ||w_k||^2  (||xn||^2 dropped:
    per-row constant, argmin-equivalent)
  - PE: u = (2*xn) @ w^T accumulated into [128n, 2048k] psum quarters
  - per quarter: ACT evicts psum, gpsimd subtracts s_rep, DVE
    max-reduces; DVE max_index recovers the first-match index
  - quarter combine with first-index tie-break (matches jnp.argmin)
  - output rows gathered from DRAM weight via indirect DMA (bit-exact rows)
"""
import sys
sys.path.insert(0, "/opt/trn_rl_repo")
import numpy as np
import concourse.bass as bass
from concourse import bacc
import concourse.mybir as mybir
from concourse.tile import TileContext
from concourse.bass_utils import run_bass_kernel_spmd

F32 = mybir.dt.float32
F32R = mybir.dt.float32r
F16 = mybir.dt.float16
I32 = mybir.dt.int32
U32 = mybir.dt.uint32
AX = mybir.AxisListType
OP = mybir.AluOpType
ACTF = mybir.ActivationFunctionType

NCORES = 8
N, K, D = 16384, 8192, 256
NS = N // NCORES            # 2048 rows per core
NT = NS // 128              # 16 n-tiles
DH = D // 128               # 2 contract halves
KQ = 2048                   # k-quarter (4 psum banks)
NQ = K // KQ                # 4 quarters
NCH = KQ // 512             # 4 matmul chunks per quarter
BN_EPS = 1e-5

# matmul numeric mode: "fp32r" | "fp16split" | "fp32"
# fp32r measured tf32-class on HW (rms ~2e-3) -> too coarse for argmin.
# fp16split (xh*wh + xh*wl + xl*wh, fp32 psum accum) reproduces the
# reference argmin exactly on the fixed dataset (verified in numpy).
MM_MODE = "fp16split"

# reduction mode: "ttr" (fused DVE) | "split" (ACT+gpsimd+DVE)
REDUCE_MODE = "split"

_cache = {}


def _build(mm_mode: str) -> bass.Bass:
    from concourse.masks import make_identity

    nc = bacc.Bacc("TRN2", target_bir_lowering=False, debug=False, num_devices=NCORES)
    x = nc.dram_tensor("x", [NS, D], F32, kind="ExternalInput")
    w = nc.dram_tensor("w", [K, D], F32, kind="ExternalInput")
    gamma = nc.dram_tensor("gamma", [D], F32, kind="ExternalInput")
    beta = nc.dram_tensor("beta", [D], F32, kind="ExternalInput")
    y = nc.dram_tensor("y", [NS, D], F32, kind="ExternalOutput")

    cc_in = nc.dram_tensor("cc_in", [128, 4], F32)
    cc_out = nc.dram_tensor("cc_out", [128, 4], F32, addr_space="Shared")
    s_dram = nc.dram_tensor("s_dram", [K], F32)

    wv = w[:, :].rearrange("(t p) d -> t p d", p=128)       # [64, 128, 256]
    xv = x[:, :].rearrange("(t p) d -> t p d", p=128)       # [16, 128, 256]
    yv = y[:, :].rearrange("(t p) d -> p t d", p=128)       # [128, 16, 256]

    mm_dt = {"fp32r": F32R, "fp32": F32, "fp16split": F16}[mm_mode]

    with TileContext(nc) as tc:
        with (
            tc.tile_pool(name="const", bufs=1) as constp,
            tc.tile_pool(name="big", bufs=1) as big,
            tc.tile_pool(name="small", bufs=1) as small,
        ):
            ident = constp.tile([128, 128], F32, tag="ident")
            make_identity(nc, ident[:, :])

            # persistent big tiles
            srep = big.tile([128, K], F32, tag="srep")
            if mm_mode == "fp16split":
                wh = [big.tile([128, K], F16, tag=f"wh{h}", name=f"wh{h}") for h in range(DH)]
                wl = [big.tile([128, K], F16, tag=f"wl{h}", name=f"wl{h}") for h in range(DH)]
                xh = [big.tile([128, NS], F16, tag=f"xh{h}", name=f"xh{h}") for h in range(DH)]
                xl = [big.tile([128, NS], F16, tag=f"xl{h}", name=f"xl{h}") for h in range(DH)]
            else:
                wT = [big.tile([128, K], mm_dt, tag=f"wT{h}", name=f"wT{h}") for h in range(DH)]
            xT = [big.tile([128, NS], F32, tag=f"xT{h}", name=f"xT{h}") for h in range(DH)]
            xn_dt = F32R if mm_mode == "fp32r" else F32
            xn2T = [big.tile([128, NS], xn_dt, tag=f"xn2T{h}", name=f"xn2T{h}") for h in range(DH)]

            s_nat = small.tile([128, K // 128], F32, tag="s_nat")   # [128, 64]
            stats = small.tile([128, 4], F32, tag="stats")
            tots = small.tile([128, 4], F32, tag="tots")
            gb = small.tile([128, 4], F32, tag="gb")
            bn = small.tile([128, 8], F32, tag="bn")
            idxf = small.tile([128, NT], F32, tag="idxf")
            idxi = small.tile([128, NT], I32, tag="idxi")
            qcand_base = small.tile([128, NQ], F32, tag="qcb")
            for q in range(NQ):
                nc.vector.memset(qcand_base[:, q : q + 1], float(q * KQ))

            # ---------- setup: load w, compute s, transpose w ----------
            with (
                tc.tile_pool(name="wload", bufs=8) as wload,
                tc.tile_pool(name="tps", bufs=8, space="PSUM") as tps,
                tc.tile_pool(name="scr", bufs=8) as scr,
                tc.tile_pool(name="scr2", bufs=2) as scr2,
            ):
                # ---------- setup: load x, transpose, BN stats ----------
                for t in range(NT):
                    xt = wload.tile([128, D], F32, tag="xt")
                    nc.sync.dma_start(out=xt, in_=xv[t])
                    for h in range(DH):
                        pt = tps.tile([128, 128], F32, tag="pt")
                        nc.tensor.transpose(pt, xt[:, h * 128 : (h + 1) * 128], ident[:, :])
                        nc.scalar.copy(
                            out=xT[h][:, t * 128 : (t + 1) * 128], in_=pt
                        )

                for h in range(DH):
                    nc.vector.tensor_reduce(
                        stats[:, h : h + 1], xT[h][:, :], axis=AX.X, op=OP.add
                    )
                    sq2 = scr2.tile([128, NS], F32, tag="sq2")
                    nc.scalar.activation(
                        out=sq2, in_=xT[h][:, :], func=ACTF.Square,
                        accum_out=stats[:, 2 + h : 3 + h],
                    )

                # ---------- AllReduce BN stats ----------
                nc.sync.dma_start(out=cc_in[:, :], in_=stats)
                nc.gpsimd.collective_compute(
                    "AllReduce", OP.add,
                    replica_groups=[list(range(NCORES))],
                    ins=[cc_in[:, :]], outs=[cc_out[:, :]],
                )
                nc.sync.dma_start(out=tots, in_=cc_out[:, :])
                for t in range(K // 128):
                    wt = wload.tile([128, D], F32, tag="wt")
                    nc.sync.dma_start(out=wt, in_=wv[t])
                    sq = scr.tile([128, D], F32, tag="sq")
                    nc.scalar.activation(
                        out=sq, in_=wt, func=ACTF.Square,
                        accum_out=s_nat[:, t : t + 1],
                    )
                    for h in range(DH):
                        pt = tps.tile([128, 128], F32, tag="pt")
                        nc.tensor.transpose(pt, wt[:, h * 128 : (h + 1) * 128], ident[:, :])
                        if mm_mode == "fp16split":
                            wtmp = scr.tile([128, 128], F32, tag="wtmp")
                            nc.scalar.copy(out=wtmp, in_=pt)
                            ksl = slice(t * 128, (t + 1) * 128)
                            nc.vector.tensor_copy(out=wh[h][:, ksl], in_=wtmp)
                            nc.gpsimd.tensor_sub(out=wl[h][:, ksl], in0=wtmp, in1=wh[h][:, ksl])
                        else:
                            nc.vector.tensor_copy(
                                out=wT[h][:, t * 128 : (t + 1) * 128], in_=pt
                            )

                # s_nat [128, 64] -> [64, 128] -> DRAM in k-order
                ps_ = tps.tile([128, 128], F32, tag="pt")
                nc.tensor.transpose(ps_[0:64, :], s_nat[:, :], ident[:, :])
                sT = scr.tile([64, 128], F32, tag="sT")
                nc.vector.tensor_copy(out=sT, in_=ps_[0:64, :])
                nc.sync.dma_start(
                    out=s_dram[:].rearrange("(t p) -> t p", p=128), in_=sT[:, :]
                )
                # broadcast DRAM s [K] -> srep [128, K]
                nc.sync.dma_start(
                    out=srep, in_=s_dram[:].unsqueeze(0).broadcast_to([128, K])
                )


                # gamma/beta -> [128, 2] each
                nc.sync.dma_start(
                    out=gb[:, 0:2], in_=gamma[:].rearrange("(h p) -> p h", p=128)
                )
                nc.sync.dma_start(
                    out=gb[:, 2:4], in_=beta[:].rearrange("(h p) -> p h", p=128)
                )

                # bn math on [128, 2] slices
                mean = bn[:, 0:2]
                var = bn[:, 2:4]
                rstd = bn[:, 4:6]
                scale2 = bn[:, 6:8]
                inv_n = 1.0 / float(N)
                nc.vector.tensor_scalar(mean, tots[:, 0:2], inv_n, scalar2=None, op0=OP.mult)
                nc.vector.tensor_scalar(var, tots[:, 2:4], inv_n, scalar2=None, op0=OP.mult)
                msq = tots[:, 0:2]
                nc.vector.tensor_tensor(out=msq, in0=mean, in1=mean, op=OP.mult)
                nc.vector.tensor_tensor(out=var, in0=var, in1=msq, op=OP.subtract)
                nc.vector.tensor_scalar(var, var, BN_EPS, scalar2=None, op0=OP.add)
                nc.vector.reciprocal(out=var, in_=var)
                nc.scalar.activation(out=rstd, in_=var, func=ACTF.Sqrt)
                # scale2 = 2*rstd*gamma ; bias2 = 2*beta - mean*scale2
                nc.vector.tensor_tensor(out=scale2, in0=rstd, in1=gb[:, 0:2], op=OP.mult)
                nc.vector.tensor_scalar(scale2, scale2, 2.0, scalar2=None, op0=OP.mult)
                bias2 = gb[:, 2:4]
                nc.vector.tensor_scalar(bias2, bias2, 2.0, scalar2=None, op0=OP.mult)
                mscale = tots[:, 0:2]
                nc.vector.tensor_tensor(out=mscale, in0=mean, in1=scale2, op=OP.mult)
                nc.vector.tensor_tensor(out=bias2, in0=bias2, in1=mscale, op=OP.subtract)

                # xn2T = xT * scale2 + bias2  (rounds to mm dtype on write)
                for h in range(DH):
                    nc.vector.tensor_scalar(
                        xn2T[h][:, :], xT[h][:, :],
                        scale2[:, h : h + 1], scalar2=bias2[:, h : h + 1],
                        op0=OP.mult, op1=OP.add,
                    )
                if mm_mode == "fp16split":
                    for h in range(DH):
                        nc.vector.tensor_copy(out=xh[h], in_=xn2T[h][:, :])
                        nc.gpsimd.tensor_sub(out=xl[h], in0=xn2T[h][:, :], in1=xh[h])

            # ---------- main loop ----------
            with (
                tc.tile_pool(name="mpsum", bufs=2, space="PSUM") as mpsum,
                tc.tile_pool(name="valp", bufs=4) as valp,
                tc.tile_pool(name="qsmall", bufs=3) as qsmall,
            ):
                for nt in range(NT):
                    nsl = slice(nt * 128, (nt + 1) * 128)
                    qmax = qsmall.tile([128, NQ], F32, tag="qmax")
                    qif = qsmall.tile([128, NQ], F32, tag="qif")
                    for q in range(NQ):
                        if True:
                            pq = mpsum.tile([128, KQ], F32, tag="pq")
                            if mm_mode == "fp16split":
                                terms = [(xh, wh), (xh, wl), (xl, wh)]
                            else:
                                terms = [(xn2T, wT)]
                            nmm = len(terms) * DH
                            # term-major within the quarter: one stationary
                            # serves 4 consecutive matmuls
                            i = 0
                            for xa, wa in terms:
                                for h in range(DH):
                                    for c in range(NCH):
                                        kofs = q * KQ + c * 512
                                        nc.tensor.matmul(
                                            pq[:, c * 512 : (c + 1) * 512],
                                            xa[h][:, nsl],
                                            wa[h][:, kofs : kofs + 512],
                                            start=(i == 0), stop=(i == nmm - 1),
                                        )
                                    i += 1
                            # val = psum - srep ; qmax[:, q] = max(val)
                            val = valp.tile([128, KQ], F32, tag="val")
                            if REDUCE_MODE == "ttr":
                                nc.vector.tensor_tensor_reduce(
                                    out=val, in0=pq, in1=srep[:, q * KQ : (q + 1) * KQ],
                                    scale=1.0, scalar=-1e30,
                                    op0=OP.subtract, op1=OP.max,
                                    accum_out=qmax[:, q : q + 1],
                                )
                            else:
                                # split: ACT evicts psum, gpsimd subtracts s,
                                # DVE reduces -- keeps DVE at 2 passes/quarter
                                nc.scalar.copy(out=val, in_=pq)
                                nc.gpsimd.tensor_sub(
                                    out=val, in0=val, in1=srep[:, q * KQ : (q + 1) * KQ]
                                )
                                nc.vector.tensor_reduce(
                                    qmax[:, q : q + 1], val, axis=AX.X, op=OP.max
                                )
                            # first-match index of the quarter max
                            i8 = qsmall.tile([128, 8], U32, tag="i8")
                            nc.vector.max_index(
                                out=i8,
                                in_max=qmax[:, q : q + 1].to_broadcast([128, 8]),
                                in_values=val,
                            )
                            nc.vector.tensor_copy(out=qif[:, q : q + 1], in_=i8[:, 0:1])

                    # combine quarters (first-index tie-break)
                    m = qsmall.tile([128, 1], F32, tag="m")
                    nc.vector.tensor_reduce(m, qmax[:, :], axis=AX.X, op=OP.max)
                    nc.vector.tensor_tensor(out=qif, in0=qif, in1=qcand_base, op=OP.add)
                    pen = qsmall.tile([128, NQ], F32, tag="pen")
                    nc.vector.tensor_scalar(
                        pen, qmax, m[:, 0:1], scalar2=1e9, op0=OP.is_lt, op1=OP.mult
                    )
                    nc.vector.tensor_tensor(out=qif, in0=qif, in1=pen, op=OP.add)
                    nc.vector.tensor_reduce(
                        idxf[:, nt : nt + 1], qif[:, :], axis=AX.X, op=OP.min
                    )
                    # gather this tile's codebook rows and emit output
                    # (one [128,1]-offset gather: multi-column offset APs
                    # land blocks in the wrong order on HW)
                    nc.vector.tensor_copy(
                        out=idxi[:, nt : nt + 1], in_=idxf[:, nt : nt + 1]
                    )
                    gath = valp.tile([128, D], F32, tag="gath")
                    nc.gpsimd.indirect_dma_start(
                        out=gath,
                        out_offset=None,
                        in_=w[:, :],
                        in_offset=bass.IndirectOffsetOnAxis(ap=idxi[:, nt : nt + 1], axis=0),
                    )
                    nc.sync.dma_start(out=yv[:, nt, :], in_=gath)



    return nc


def _get_nc():
    key = (MM_MODE, REDUCE_MODE)
    if key not in _cache:
        nc_ = _build(MM_MODE)
        if not nc_.is_finalized():
            nc_.finalize()
        _cache[key] = nc_
    return _cache[key]


def kernel(x, weight, gamma, beta):
    x = np.ascontiguousarray(x, dtype=np.float32)
    weight = np.ascontiguousarray(weight, dtype=np.float32)
    gamma = np.ascontiguousarray(gamma, dtype=np.float32)
    beta = np.ascontiguousarray(beta, dtype=np.float32)

    nc = _get_nc()
    in_maps = [
        {
            "x": x[c * NS : (c + 1) * NS],
            "w": weight,
            "gamma": gamma,
            "beta": beta,
        }
        for c in range(NCORES)
    ]
    res = run_bass_kernel_spmd(nc, in_maps, list(range(NCORES)))
    return np.concatenate([res.results[c]["y"] for c in range(NCORES)], axis=0)


if __name__ == "__main__":
    _build(MM_MODE)
    print("kernel build OK")
